# revision 6
# baseline (speedup 1.0000x reference)
"""Multi-head attention (B=2, L=2048, H=16, D=64) on 8 TRN2 NeuronCores.

Sharding: core = (batch b, head-group hg); 2 batches x 4 groups of 4 heads.
All matmul inputs are bf16 (hardware runs bf16 moving data at 1 row/cycle
@2.4GHz vs 2 rows/cycle for f32r); PSUM accumulation stays f32.

Structure: 8 units = (head-pair m, i-512 block q); per unit a 16-step j-loop:
    S^T pair: two K=64 matmuls (PE row groups 0 / 64, adjacent in pc so the
              hardware can overlap them) -> PSUM supertile
              [128 j, {h0 i-512 | h1 i-512}]
    exp:      ONE ACTIVATE [128, 1024] PSUM->SBUF bf16. The scalar engine is
              the kernel roofline: 128 instrs x ~1.11us = ~143us.
    AV(j-1):  two M=65 matmuls (V|ones column) accumulating O'^T + softmax
              denominator row (software-pipelined one step behind exp)
    + interleaved projection/output items pumped from a deadline queue to
      fill PE slack under the ACT roofline: qk projections for later units,
      Wo output chunks for finished i-blocks, deferred normalizes.

Prologue: qk projection for (m0, n0) + all V (both pairs, N=256) overlapped
with the streaming xT DMA.  Epilogue: last normalize + Wo(q3) chunks.
"""

import sys

try:
    import concourse.bass as bass  # noqa: F401
except ImportError:  # pragma: no cover - path fallback
    sys.path.insert(0, "/opt/trn_rl_repo")

import numpy as np
import ml_dtypes
import concourse.bass as bass
import concourse.mybir as mybir
import concourse.tile as tile
from concourse import bacc
from concourse.bass_utils import run_bass_kernel_spmd

F32 = mybir.dt.float32
BF16 = mybir.dt.bfloat16
AF = mybir.ActivationFunctionType

B = 2
L = 2048          # sequence length
C = 1024          # model dim
H_LOC = 4         # heads per core
D = 64            # head dim
HD = H_LOC * D    # 256 = local head-group width
KT = C // 128     # 8 k-tiles over the model dim
SCALE2 = float(D) ** -0.5  # 1/8, applied once inside exp

_cache = {}


def _build():
    nc = bacc.Bacc("TRN2", target_bir_lowering=False, debug=False, num_devices=8)

    xT = nc.declare_dram_parameter("xT", [C, L], BF16, isOutput=False)
    wq = nc.declare_dram_parameter("wq", [C, HD], BF16, isOutput=False)
    wk = nc.declare_dram_parameter("wk", [C, HD], BF16, isOutput=False)
    wv = nc.declare_dram_parameter("wv", [C, HD], BF16, isOutput=False)
    wo = nc.declare_dram_parameter("wo", [HD, C], BF16, isOutput=False)
    outT = nc.declare_dram_parameter("outT", [C, L], F32, isOutput=True)

    with tile.TileContext(nc) as tc:
        with tc.tile_pool(name="sb", bufs=1) as sb, \
             tc.tile_pool(name="sp", bufs=2, space="PSUM") as sp, \
             tc.tile_pool(name="op", bufs=2, space="PSUM") as op, \
             tc.tile_pool(name="pp", bufs=2, space="PSUM") as pp:

            es_pool = tc.alloc_tile_pool(name="es_pool", bufs=4)
            ocp_pool = tc.alloc_tile_pool(name="ocp_pool", bufs=4)
            nrm_pool = tc.alloc_tile_pool(name="nrm_pool", bufs=2)
            ost_pool = tc.alloc_tile_pool(name="ost_pool", bufs=3)

            # ---- input DMAs (wq/wk first so the first projection starts
            # early; xT per k-tile so matmuls chase the stream) -------------
            wq_sb = sb.tile([128, KT, HD], BF16, tag="wq")
            wk_sb = sb.tile([128, KT, HD], BF16, tag="wk")
            wv_sb = sb.tile([128, KT, HD], BF16, tag="wv")
            wo_sb = sb.tile([128, 2, C], BF16, tag="wo")
            xT_sb = sb.tile([128, KT, L], BF16, tag="xT")
            nc.sync.dma_start(wq_sb[:, :, :], wq.rearrange("(k p) c -> p k c", p=128))
            nc.sync.dma_start(wk_sb[:, :, :], wk.rearrange("(k p) c -> p k c", p=128))
            for n in range(4):
                for k in range(KT):
                    nc.sync.dma_start(
                        xT_sb[:, k, n * 512:(n + 1) * 512],
                        xT[k * 128:(k + 1) * 128, n * 512:(n + 1) * 512])
            nc.sync.dma_start(wv_sb[:, :, :], wv.rearrange("(k p) c -> p k c", p=128))
            nc.sync.dma_start(wo_sb[:, :, :], wo.rearrange("(k p) c -> p k c", p=128))

            ones_f = sb.tile([128, 64], F32, tag="ones_f")
            nc.vector.memset(ones_f[:], 1.0)

            qT_sb = sb.tile([128, 2, L], BF16, tag="qT")
            kT_sb = sb.tile([128, 2, L], BF16, tag="kT")
            v_sb = sb.tile([128, 16, H_LOC, D + 1], BF16, tag="v")
            oT_sb = sb.tile([128, 2, L], BF16, tag="oT")

            nc.vector.tensor_copy(
                v_sb[:, :, :, D:D + 1],
                ones_f.rearrange("p (a b c) -> p a b c", a=16, b=4),
            )

            # ---- projection / output emitters ------------------------------
            def emit_qk_chunk(w_sb, t_sb, m, n):
                # one [128 d-pair, 512 i] chunk of Q^T or K^T = W^T x^T
                p = pp.tile([128, 512], F32, tag="p", name="qk_ps")
                for k in range(KT):
                    nc.tensor.matmul(
                        p[:, :],
                        w_sb[:, k, m * 128:(m + 1) * 128],
                        xT_sb[:, k, n * 512:(n + 1) * 512],
                        start=(k == 0), stop=(k == KT - 1),
                    )
                nc.vector.tensor_copy(t_sb[:, m, n * 512:(n + 1) * 512], p[:, :])

            def emit_v_chunk(it):
                # V rows for j-tile `it`, all 4 heads at once (N=256)
                p = pp.tile([128, 512], F32, tag="p", name="v_ps")
                acc = p[:, 0:HD]
                for k in range(KT):
                    nc.tensor.matmul(
                        acc,
                        xT_sb[:, k, it * 128:(it + 1) * 128],
                        wv_sb[:, k, :],
                        start=(k == 0), stop=(k == KT - 1),
                    )
                nc.vector.tensor_copy(
                    v_sb[:, it, :, 0:D],
                    acc.rearrange("p (h d) -> p h d", h=H_LOC),
                )

            def emit_wo_chunk(ct, q):
                # [128 ct, 512 i] output chunk (contraction over HD=256)
                p = pp.tile([128, 512], F32, tag="p", name="wo_ps")
                for kk in range(2):
                    nc.tensor.matmul(
                        p[:, :],
                        wo_sb[:, kk, ct * 128:(ct + 1) * 128],
                        oT_sb[:, kk, q * 512:(q + 1) * 512],
                        start=(kk == 0), stop=(kk == 1),
                    )
                ost = ost_pool.tile([128, 512], F32, tag="ost", name="ost")
                nc.vector.tensor_copy(ost[:], p[:, :])
                nc.sync.dma_start(
                    outT[ct * 128:(ct + 1) * 128, q * 512:(q + 1) * 512], ost[:])

            # ---- deadline-driven work queue for PE slack -------------------
            # items: [deadline_step, cost_ns, emit_fn]; pumped each j-step
            work = []

            def pump(s, avail):
                while work and work[0][0] <= s:
                    _, c, f = work.pop(0)
                    f()
                    avail -= c
                while work and avail > 0:
                    idx = None
                    for i, (_, c, _f) in enumerate(work):
                        if c <= avail + 150.0:
                            idx = i
                            break
                    if idx is None:
                        break
                    _, c, f = work.pop(idx)
                    f()
                    avail -= c
                return avail

            QK_COST = 1750.0
            V_COST = 1000.0
            WO_COST = 500.0

            # Deadlines (in j-steps): every unit sweeps the FULL kT range
            # (j = 0..2048), so ALL kT chunks for pair m must be emitted well
            # before unit (m, 0) starts. qT chunk n is only read by unit
            # (m, q=n) -- give it ~8 steps of runway. V chunk `it` is read by
            # the AV emitted at step it+1.
            for it in range(16):
                work.append([it - 1.0, V_COST,
                             lambda it=it: emit_v_chunk(it)])
            for n in range(1, 4):
                work.append([float(n), QK_COST,
                             lambda n=n: emit_qk_chunk(wk_sb, kT_sb, 0, n)])
            for n in range(1, 4):
                work.append([16.0 * n - 8.0, QK_COST,
                             lambda n=n: emit_qk_chunk(wq_sb, qT_sb, 0, n)])
            for i, n in enumerate(range(4)):
                work.append([40.0 + 4 * i, QK_COST,
                             lambda n=n: emit_qk_chunk(wk_sb, kT_sb, 1, n)])
            for n in range(4):
                work.append([56.0 + 16 * n, QK_COST,
                             lambda n=n: emit_qk_chunk(wq_sb, qT_sb, 1, n)])
            work.sort(key=lambda item: item[0])

            # ---- prologue: just q/k (m0, n0), chasing the n0 DMA stream ----
            emit_qk_chunk(wq_sb, qT_sb, 0, 0)
            emit_qk_chunk(wk_sb, kT_sb, 0, 0)

            # ---- attention units ------------------------------------------
            # finished units queue normalize closures; one runs per j-step.
            norm_steps = []

            def queue_normalize(m, q, o_cps):
                i0 = q * 512
                d0s = [None, None]

                def recip(hl):
                    d0 = nrm_pool.tile([1, 512], F32, tag=f"d0_{hl}",
                                       name=f"d0_{hl}")
                    nc.vector.reciprocal(d0[:], o_cps[hl][64:65, :])
                    d0s[hl] = d0

                def scale(hl):
                    rep = nrm_pool.tile([64, 512], F32, tag=f"rep{hl}",
                                        name=f"rep{hl}")
                    nc.gpsimd.partition_broadcast(rep[:], d0s[hl][:])
                    with nc.allow_low_precision(reason="bf16 oT"):
                        if hl == 0:
                            nc.vector.tensor_mul(
                                oT_sb[0:64, m, i0:i0 + 512],
                                o_cps[hl][0:64, :], rep[:])
                        else:
                            stage = nrm_pool.tile([64, 512], BF16, tag="stage",
                                                  name="stage")
                            nc.vector.tensor_mul(
                                stage[:], o_cps[hl][0:64, :], rep[:])
                            nc.gpsimd.dma_start(
                                oT_sb[64:128, m, i0:i0 + 512], stage[:])

                norm_steps.append(lambda: recip(0))
                norm_steps.append(lambda: recip(1))
                norm_steps.append(lambda: scale(0))
                norm_steps.append(lambda: scale(1))
                if m == 1:
                    # oT for this i-block complete after both scales -> Wo
                    def queue_wo():
                        for ct in range(8):
                            work.append([10 ** 9, WO_COST,
                                         lambda ct=ct, q=q: emit_wo_chunk(ct, q)])
                    norm_steps.append(queue_wo)

            units = [(0, q) for q in range(4)] + [(1, q) for q in range(4)]
            for ui, (m, q) in enumerate(units):
                i0 = q * 512
                acc_h = [None, None]
                es_prev = None
                for j in range(16):
                    s = 16 * ui + j
                    # S^T pair -> supertile (adjacent pc: row groups 0 / 64)
                    spt = sp.tile([128, 1024], F32, tag="s", name="spt")
                    nc.tensor.matmul(
                        spt[:, 0:512],
                        kT_sb[0:64, m, j * 128:(j + 1) * 128],
                        qT_sb[0:64, m, i0:i0 + 512],
                        start=True, stop=True,
                    )
                    nc.tensor.matmul(
                        spt[:, 512:1024],
                        kT_sb[64:128, m, j * 128:(j + 1) * 128],
                        qT_sb[64:128, m, i0:i0 + 512],
                        start=True, stop=True,
                    )
                    es = es_pool.tile([128, 1024], BF16, tag="es", name="es")
                    nc.scalar.activation(es[:], spt[:], AF.Exp, scale=SCALE2)
                    # AV for step j-1 (one step behind)
                    if j > 0:
                        if j == 1:
                            acc_h[0] = op.tile([128, 512], F32, tag="o",
                                               name="acc0")
                            acc_h[1] = op.tile([128, 512], F32, tag="o",
                                               name="acc1")
                        for hl in range(2):
                            nc.tensor.matmul(
                                acc_h[hl][0:65, :],
                                v_sb[:, j - 1, 2 * m + hl, :],
                                es_prev[:, hl * 512:(hl + 1) * 512],
                                start=(j == 1), stop=False,
                            )
                    if norm_steps:
                        norm_steps.pop(0)()
                    pump(s, 600.0)
                    es_prev = es
                # epilogue AV for j=15
                for hl in range(2):
                    nc.tensor.matmul(
                        acc_h[hl][0:65, :],
                        v_sb[:, 15, 2 * m + hl, :],
                        es_prev[:, hl * 512:(hl + 1) * 512],
                        start=False, stop=True,
                    )
                # pull O' + denominator off PSUM so the banks free quickly
                o_cps = []
                for hl in range(2):
                    o_cp = ocp_pool.tile([65, 512], F32, tag=f"ocp{hl}",
                                         name=f"ocp{hl}")
                    nc.vector.tensor_copy(o_cp[:], acc_h[hl][0:65, :])
                    o_cps.append(o_cp)
                queue_normalize(m, q, o_cps)

            # ---- epilogue: last normalize + remaining Wo -------------------
            while norm_steps:
                norm_steps.pop(0)()
            while work:
                work.pop(0)[2]()

            ost_pool.release()
            nrm_pool.release()
            ocp_pool.release()
            es_pool.release()

    nc.compile()
    return nc


def kernel(x, Wq, Wk, Wv, Wo, bo):
    x = np.asarray(x, dtype=np.float32)
    Wq = np.asarray(Wq, dtype=np.float32)
    Wk = np.asarray(Wk, dtype=np.float32)
    Wv = np.asarray(Wv, dtype=np.float32)
    Wo = np.asarray(Wo, dtype=np.float32)
    bo = np.asarray(bo, dtype=np.float32)

    if "nc" not in _cache:
        _cache["nc"] = _build()
    nc = _cache["nc"]

    xTs = [np.ascontiguousarray(x[b].T) for b in range(B)]
    in_maps = []
    for core in range(8):
        b, hg = divmod(core, 4)
        sl = slice(hg * HD, (hg + 1) * HD)
        in_maps.append({
            "xT": xTs[b].astype(ml_dtypes.bfloat16),
            "wq": np.ascontiguousarray(Wq[:, sl]).astype(ml_dtypes.bfloat16),
            "wk": np.ascontiguousarray(Wk[:, sl]).astype(ml_dtypes.bfloat16),
            "wv": np.ascontiguousarray(Wv[:, sl]).astype(ml_dtypes.bfloat16),
            "wo": np.ascontiguousarray(Wo[sl, :]).astype(ml_dtypes.bfloat16),
        })

    res = run_bass_kernel_spmd(nc, in_maps, core_ids=list(range(8)))
    out = np.empty((B, L, C), dtype=np.float32)
    for b in range(B):
        acc = res.results[4 * b]["outT"]
        for hg in range(1, 4):
            acc = acc + res.results[4 * b + hg]["outT"]
        out[b] = acc.T + bo
    return out


# revision 7
# speedup vs baseline: 1.1030x; 1.1030x over previous
"""Multi-head attention (B=2, L=2048, H=16, D=64) on 8 TRN2 NeuronCores.

Sharding: core = (batch b, head-group hg); 2 batches x 4 groups of 4 heads.
All matmul inputs are bf16 (hardware runs bf16 moving data at 1 row/cycle
@2.4GHz vs 2 rows/cycle for f32r); PSUM accumulation stays f32.

Structure: 8 units = (head-pair m, i-512 block q); per unit a 16-step j-loop:
    S^T pair: two K=64 matmuls (PE row groups 0 / 64, adjacent in pc so the
              hardware can overlap them) -> PSUM supertile
              [128 j, {h0 i-512 | h1 i-512}]
    exp:      ONE ACTIVATE [128, 1024] PSUM->SBUF bf16. The scalar engine is
              the kernel roofline: 128 instrs x ~1.11us = ~143us.
    AV(j-1):  two M=65 matmuls (V|ones column) accumulating O'^T + softmax
              denominator row (software-pipelined one step behind exp)
    + interleaved projection/output items pumped from a deadline queue to
      fill PE slack under the ACT roofline: qk projections for later units,
      Wo output chunks for finished i-blocks, deferred normalizes.

Prologue: qk projection for (m0, n0) + all V (both pairs, N=256) overlapped
with the streaming xT DMA.  Epilogue: last normalize + Wo(q3) chunks.
"""

import sys

try:
    import concourse.bass as bass  # noqa: F401
except ImportError:  # pragma: no cover - path fallback
    sys.path.insert(0, "/opt/trn_rl_repo")

import numpy as np
import ml_dtypes
import concourse.bass as bass
import concourse.mybir as mybir
import concourse.tile as tile
from concourse import bacc
from concourse.bass_utils import run_bass_kernel_spmd

F32 = mybir.dt.float32
BF16 = mybir.dt.bfloat16
AF = mybir.ActivationFunctionType

B = 2
L = 2048          # sequence length
C = 1024          # model dim
H_LOC = 4         # heads per core
D = 64            # head dim
HD = H_LOC * D    # 256 = local head-group width
KT = C // 128     # 8 k-tiles over the model dim
SCALE2 = float(D) ** -0.5  # 1/8, applied once inside exp

_cache = {}


def _build():
    nc = bacc.Bacc("TRN2", target_bir_lowering=False, debug=False, num_devices=8)

    xT = nc.declare_dram_parameter("xT", [C, L], BF16, isOutput=False)
    wq = nc.declare_dram_parameter("wq", [128, KT * HD], BF16, isOutput=False)
    wk = nc.declare_dram_parameter("wk", [128, KT * HD], BF16, isOutput=False)
    wv = nc.declare_dram_parameter("wv", [128, KT * HD], BF16, isOutput=False)
    wo = nc.declare_dram_parameter("wo", [128, 2 * C], BF16, isOutput=False)
    outT = nc.declare_dram_parameter("outT", [C, L], F32, isOutput=True)

    with tile.TileContext(nc) as tc:
        with tc.tile_pool(name="sb", bufs=1) as sb, \
             tc.tile_pool(name="sp", bufs=2, space="PSUM") as sp, \
             tc.tile_pool(name="op", bufs=2, space="PSUM") as op, \
             tc.tile_pool(name="pp", bufs=2, space="PSUM") as pp:

            es_pool = tc.alloc_tile_pool(name="es_pool", bufs=4)
            ocp_pool = tc.alloc_tile_pool(name="ocp_pool", bufs=4)
            nrm_pool = tc.alloc_tile_pool(name="nrm_pool", bufs=2)
            ost_pool = tc.alloc_tile_pool(name="ost_pool", bufs=3)

            # ---- input DMAs (wq/wk first so the first projection starts
            # early; xT per k-tile so matmuls chase the stream) -------------
            wq_sb = sb.tile([128, KT, HD], BF16, tag="wq")
            wk_sb = sb.tile([128, KT, HD], BF16, tag="wk")
            wv_sb = sb.tile([128, KT, HD], BF16, tag="wv")
            wo_sb = sb.tile([128, 2, C], BF16, tag="wo")
            xT_sb = sb.tile([128, KT, L], BF16, tag="xT")
            nc.sync.dma_start(wq_sb[:, :, :], wq.rearrange("p (k c) -> p k c", k=KT))
            nc.sync.dma_start(wk_sb[:, :, :], wk.rearrange("p (k c) -> p k c", k=KT))
            for n in range(4):
                for k in range(KT):
                    nc.sync.dma_start(
                        xT_sb[:, k, n * 512:(n + 1) * 512],
                        xT[k * 128:(k + 1) * 128, n * 512:(n + 1) * 512])
            nc.sync.dma_start(wv_sb[:, :, :], wv.rearrange("p (k c) -> p k c", k=KT))
            nc.sync.dma_start(wo_sb[:, :, :], wo.rearrange("p (k c) -> p k c", k=2))

            ones_f = sb.tile([128, 64], F32, tag="ones_f")
            nc.vector.memset(ones_f[:], 1.0)

            qT_sb = sb.tile([128, 2, L], BF16, tag="qT")
            kT_sb = sb.tile([128, 2, L], BF16, tag="kT")
            v_sb = sb.tile([128, 16, H_LOC, D + 1], BF16, tag="v")
            oT_sb = sb.tile([128, 2, L], BF16, tag="oT")

            nc.vector.tensor_copy(
                v_sb[:, :, :, D:D + 1],
                ones_f.rearrange("p (a b c) -> p a b c", a=16, b=4),
            )

            # ---- projection / output emitters ------------------------------
            def emit_qk_chunk(w_sb, t_sb, m, n):
                # one [128 d-pair, 512 i] chunk of Q^T or K^T = W^T x^T
                p = pp.tile([128, 512], F32, tag="p", name="qk_ps")
                for k in range(KT):
                    nc.tensor.matmul(
                        p[:, :],
                        w_sb[:, k, m * 128:(m + 1) * 128],
                        xT_sb[:, k, n * 512:(n + 1) * 512],
                        start=(k == 0), stop=(k == KT - 1),
                    )
                nc.vector.tensor_copy(t_sb[:, m, n * 512:(n + 1) * 512], p[:, :])

            def emit_v_chunk(it):
                # V rows for j-tile `it`, all 4 heads at once (N=256)
                p = pp.tile([128, 512], F32, tag="p", name="v_ps")
                acc = p[:, 0:HD]
                for k in range(KT):
                    nc.tensor.matmul(
                        acc,
                        xT_sb[:, k, it * 128:(it + 1) * 128],
                        wv_sb[:, k, :],
                        start=(k == 0), stop=(k == KT - 1),
                    )
                nc.vector.tensor_copy(
                    v_sb[:, it, :, 0:D],
                    acc.rearrange("p (h d) -> p h d", h=H_LOC),
                )

            def emit_wo_chunk(ct, q):
                # [128 ct, 512 i] output chunk (contraction over HD=256)
                p = pp.tile([128, 512], F32, tag="p", name="wo_ps")
                for kk in range(2):
                    nc.tensor.matmul(
                        p[:, :],
                        wo_sb[:, kk, ct * 128:(ct + 1) * 128],
                        oT_sb[:, kk, q * 512:(q + 1) * 512],
                        start=(kk == 0), stop=(kk == 1),
                    )
                ost = ost_pool.tile([128, 512], F32, tag="ost", name="ost")
                nc.vector.tensor_copy(ost[:], p[:, :])
                nc.sync.dma_start(
                    outT[ct * 128:(ct + 1) * 128, q * 512:(q + 1) * 512], ost[:])

            # ---- deadline-driven work queue for PE slack -------------------
            # items: [deadline_step, cost_ns, emit_fn]; pumped each j-step
            work = []

            def pump(s, avail):
                while work and work[0][0] <= s:
                    _, c, f = work.pop(0)
                    f()
                    avail -= c
                while work and avail > 0:
                    idx = None
                    for i, (_, c, _f) in enumerate(work):
                        if c <= avail + 150.0:
                            idx = i
                            break
                    if idx is None:
                        break
                    _, c, f = work.pop(idx)
                    f()
                    avail -= c
                return avail

            QK_COST = 1750.0
            V_COST = 1000.0
            WO_COST = 500.0

            # Deadlines (in j-steps): every unit sweeps the FULL kT range
            # (j = 0..2048), so ALL kT chunks for pair m must be emitted well
            # before unit (m, 0) starts. qT chunk n is only read by unit
            # (m, q=n) -- give it ~8 steps of runway. V chunk `it` is read by
            # the AV emitted at step it+1.
            for it in range(4, 16):
                work.append([it - 1.0, V_COST,
                             lambda it=it: emit_v_chunk(it)])
            for n in range(1, 4):
                work.append([float(n), QK_COST,
                             lambda n=n: emit_qk_chunk(wk_sb, kT_sb, 0, n)])
            for n in range(1, 4):
                work.append([16.0 * n - 8.0, QK_COST,
                             lambda n=n: emit_qk_chunk(wq_sb, qT_sb, 0, n)])
            for i, n in enumerate(range(4)):
                work.append([40.0 + 4 * i, QK_COST,
                             lambda n=n: emit_qk_chunk(wk_sb, kT_sb, 1, n)])
            for n in range(4):
                work.append([56.0 + 16 * n, QK_COST,
                             lambda n=n: emit_qk_chunk(wq_sb, qT_sb, 1, n)])
            work.sort(key=lambda item: item[0])

            # ---- prologue: just q/k (m0, n0), chasing the n0 DMA stream ----
            emit_qk_chunk(wq_sb, qT_sb, 0, 0)
            emit_qk_chunk(wk_sb, kT_sb, 0, 0)
            for it in range(4):
                emit_v_chunk(it)

            # ---- attention units ------------------------------------------
            # finished units queue normalize closures; one runs per j-step.
            norm_steps = []

            def queue_normalize(m, q, o_cps):
                i0 = q * 512
                d0s = [None, None]

                def recip(hl):
                    # reshape the denominator row across all 128 lanes so the
                    # reciprocal runs at full DVE width, then reshape back
                    dsq = nrm_pool.tile([128, 4], F32, tag=f"dsq{hl}",
                                        name=f"dsq{hl}")
                    nc.gpsimd.dma_start(dsq[:], o_cps[hl][64:65, :])
                    nc.vector.reciprocal(dsq[:], dsq[:])
                    d0 = nrm_pool.tile([1, 512], F32, tag=f"d0_{hl}",
                                       name=f"d0_{hl}")
                    nc.gpsimd.dma_start(d0[:], dsq[:])
                    d0s[hl] = d0

                def scale(hl):
                    rep = nrm_pool.tile([64, 512], F32, tag=f"rep{hl}",
                                        name=f"rep{hl}")
                    nc.gpsimd.partition_broadcast(rep[:], d0s[hl][:])
                    with nc.allow_low_precision(reason="bf16 oT"):
                        if hl == 0:
                            nc.vector.tensor_mul(
                                oT_sb[0:64, m, i0:i0 + 512],
                                o_cps[hl][0:64, :], rep[:])
                        else:
                            stage = nrm_pool.tile([64, 512], BF16, tag="stage",
                                                  name="stage")
                            nc.vector.tensor_mul(
                                stage[:], o_cps[hl][0:64, :], rep[:])
                            nc.gpsimd.dma_start(
                                oT_sb[64:128, m, i0:i0 + 512], stage[:])

                norm_steps.append(lambda: recip(0))
                norm_steps.append(lambda: recip(1))
                norm_steps.append(lambda: scale(0))
                norm_steps.append(lambda: scale(1))
                if m == 1:
                    # oT for this i-block complete after both scales -> Wo
                    def queue_wo():
                        for ct in range(8):
                            work.append([10 ** 9, WO_COST,
                                         lambda ct=ct, q=q: emit_wo_chunk(ct, q)])
                    norm_steps.append(queue_wo)

            units = [(0, q) for q in range(4)] + [(1, q) for q in range(4)]
            for ui, (m, q) in enumerate(units):
                i0 = q * 512
                acc_h = [None, None]
                es_prev = None
                for j in range(16):
                    s = 16 * ui + j
                    # S^T pair -> supertile (adjacent pc: row groups 0 / 64)
                    spt = sp.tile([128, 1024], F32, tag="s", name="spt")
                    nc.tensor.matmul(
                        spt[:, 0:512],
                        kT_sb[0:64, m, j * 128:(j + 1) * 128],
                        qT_sb[0:64, m, i0:i0 + 512],
                        start=True, stop=True,
                    )
                    nc.tensor.matmul(
                        spt[:, 512:1024],
                        kT_sb[64:128, m, j * 128:(j + 1) * 128],
                        qT_sb[64:128, m, i0:i0 + 512],
                        start=True, stop=True,
                    )
                    es = es_pool.tile([128, 1024], BF16, tag="es", name="es")
                    nc.scalar.activation(es[:], spt[:], AF.Exp, scale=SCALE2)
                    # AV for step j-1 (one step behind)
                    if j > 0:
                        if j == 1:
                            acc_h[0] = op.tile([128, 512], F32, tag="o",
                                               name="acc0")
                            acc_h[1] = op.tile([128, 512], F32, tag="o",
                                               name="acc1")
                        for hl in range(2):
                            nc.tensor.matmul(
                                acc_h[hl][0:65, :],
                                v_sb[:, j - 1, 2 * m + hl, :],
                                es_prev[:, hl * 512:(hl + 1) * 512],
                                start=(j == 1), stop=False,
                            )
                    if norm_steps:
                        norm_steps.pop(0)()
                    pump(s, 600.0)
                    es_prev = es
                # epilogue AV for j=15
                for hl in range(2):
                    nc.tensor.matmul(
                        acc_h[hl][0:65, :],
                        v_sb[:, 15, 2 * m + hl, :],
                        es_prev[:, hl * 512:(hl + 1) * 512],
                        start=False, stop=True,
                    )
                # pull O' + denominator off PSUM so the banks free quickly
                o_cps = []
                for hl in range(2):
                    o_cp = ocp_pool.tile([65, 512], F32, tag=f"ocp{hl}",
                                         name=f"ocp{hl}")
                    nc.vector.tensor_copy(o_cp[:], acc_h[hl][0:65, :])
                    o_cps.append(o_cp)
                queue_normalize(m, q, o_cps)

            # ---- epilogue: last normalize + remaining Wo -------------------
            while norm_steps:
                norm_steps.pop(0)()
            while work:
                work.pop(0)[2]()

            ost_pool.release()
            nrm_pool.release()
            ocp_pool.release()
            es_pool.release()

    nc.compile()
    return nc


def kernel(x, Wq, Wk, Wv, Wo, bo):
    x = np.asarray(x, dtype=np.float32)
    Wq = np.asarray(Wq, dtype=np.float32)
    Wk = np.asarray(Wk, dtype=np.float32)
    Wv = np.asarray(Wv, dtype=np.float32)
    Wo = np.asarray(Wo, dtype=np.float32)
    bo = np.asarray(bo, dtype=np.float32)

    if "nc" not in _cache:
        _cache["nc"] = _build()
    nc = _cache["nc"]

    xTs = [np.ascontiguousarray(x[b].T) for b in range(B)]
    in_maps = []
    for core in range(8):
        b, hg = divmod(core, 4)
        sl = slice(hg * HD, (hg + 1) * HD)
        def pkc(w):
            # [(k p), c] -> [p, (k c)] so the on-chip DMA is contiguous
            kk, cc = w.shape[0] // 128, w.shape[1]
            return np.ascontiguousarray(
                w.reshape(kk, 128, cc).transpose(1, 0, 2).reshape(128, kk * cc)
            ).astype(ml_dtypes.bfloat16)

        in_maps.append({
            "xT": xTs[b].astype(ml_dtypes.bfloat16),
            "wq": pkc(Wq[:, sl]),
            "wk": pkc(Wk[:, sl]),
            "wv": pkc(Wv[:, sl]),
            "wo": pkc(Wo[sl, :]),
        })

    res = run_bass_kernel_spmd(nc, in_maps, core_ids=list(range(8)))
    out = np.empty((B, L, C), dtype=np.float32)
    for b in range(B):
        acc = res.results[4 * b]["outT"]
        for hg in range(1, 4):
            acc = acc + res.results[4 * b + hg]["outT"]
        out[b] = acc.T + bo
    return out


# revision 8
# speedup vs baseline: 1.1337x; 1.0279x over previous
"""Multi-head attention (B=2, L=2048, H=16, D=64) on 8 TRN2 NeuronCores.

Sharding: core = (batch b, head-group hg); 2 batches x 4 groups of 4 heads.
All matmul inputs are bf16 (hardware runs bf16 moving data at 1 row/cycle
@2.4GHz vs 2 rows/cycle for f32r); PSUM accumulation stays f32.

Structure: 8 units = (head-pair m, i-512 block q); per unit a 16-step j-loop:
    S^T pair: two K=64 matmuls (PE row groups 0 / 64, adjacent in pc so the
              hardware can overlap them) -> PSUM supertile
              [128 j, {h0 i-512 | h1 i-512}]
    exp:      ONE ACTIVATE [128, 1024] PSUM->SBUF bf16. The scalar engine is
              the kernel roofline: 128 instrs x ~1.11us = ~143us.
    AV(j-1):  two M=65 matmuls (V|ones column) accumulating O'^T + softmax
              denominator row (software-pipelined one step behind exp)
    + interleaved projection/output items pumped from a deadline queue to
      fill PE slack under the ACT roofline: qk projections for later units,
      Wo output chunks for finished i-blocks, deferred normalizes.

Prologue: qk projection for (m0, n0) + all V (both pairs, N=256) overlapped
with the streaming xT DMA.  Epilogue: last normalize + Wo(q3) chunks.
"""

import sys

try:
    import concourse.bass as bass  # noqa: F401
except ImportError:  # pragma: no cover - path fallback
    sys.path.insert(0, "/opt/trn_rl_repo")

import numpy as np
import ml_dtypes
import concourse.bass as bass
import concourse.mybir as mybir
import concourse.tile as tile
from concourse import bacc
from concourse.bass_utils import run_bass_kernel_spmd

F32 = mybir.dt.float32
BF16 = mybir.dt.bfloat16
AF = mybir.ActivationFunctionType

B = 2
L = 2048          # sequence length
C = 1024          # model dim
H_LOC = 4         # heads per core
D = 64            # head dim
HD = H_LOC * D    # 256 = local head-group width
KT = C // 128     # 8 k-tiles over the model dim
SCALE2 = float(D) ** -0.5  # 1/8, applied once inside exp

_cache = {}


def _build():
    nc = bacc.Bacc("TRN2", target_bir_lowering=False, debug=False, num_devices=8)

    xT = nc.declare_dram_parameter("xT", [C, L], BF16, isOutput=False)
    wq = nc.declare_dram_parameter("wq", [128, KT * HD], BF16, isOutput=False)
    wk = nc.declare_dram_parameter("wk", [128, KT * HD], BF16, isOutput=False)
    wv = nc.declare_dram_parameter("wv", [128, KT * HD], BF16, isOutput=False)
    wo = nc.declare_dram_parameter("wo", [128, 2 * C], BF16, isOutput=False)
    outT = nc.declare_dram_parameter("outT", [C, L], F32, isOutput=True)

    with tile.TileContext(nc) as tc:
        with tc.tile_pool(name="sb", bufs=1) as sb, \
             tc.tile_pool(name="sp", bufs=2, space="PSUM") as sp, \
             tc.tile_pool(name="op", bufs=2, space="PSUM") as op, \
             tc.tile_pool(name="pp", bufs=2, space="PSUM") as pp:

            es_pool = tc.alloc_tile_pool(name="es_pool", bufs=4)
            ocp_pool = tc.alloc_tile_pool(name="ocp_pool", bufs=4)
            nrm_pool = tc.alloc_tile_pool(name="nrm_pool", bufs=2)
            ost_pool = tc.alloc_tile_pool(name="ost_pool", bufs=4)

            # ---- input DMAs (wq/wk first so the first projection starts
            # early; xT per k-tile so matmuls chase the stream) -------------
            wq_sb = sb.tile([128, KT, HD], BF16, tag="wq")
            wk_sb = sb.tile([128, KT, HD], BF16, tag="wk")
            wv_sb = sb.tile([128, KT, HD], BF16, tag="wv")
            wo_sb = sb.tile([128, 2, C], BF16, tag="wo")
            xT_sb = sb.tile([128, KT, L], BF16, tag="xT")
            nc.sync.dma_start(wq_sb[:, :, :], wq.rearrange("p (k c) -> p k c", k=KT))
            nc.sync.dma_start(wk_sb[:, :, :], wk.rearrange("p (k c) -> p k c", k=KT))
            for n in range(4):
                for k in range(KT):
                    nc.sync.dma_start(
                        xT_sb[:, k, n * 512:(n + 1) * 512],
                        xT[k * 128:(k + 1) * 128, n * 512:(n + 1) * 512])
            nc.sync.dma_start(wv_sb[:, :, :], wv.rearrange("p (k c) -> p k c", k=KT))
            nc.sync.dma_start(wo_sb[:, :, :], wo.rearrange("p (k c) -> p k c", k=2))

            ones_f = sb.tile([128, 64], F32, tag="ones_f")
            nc.vector.memset(ones_f[:], 1.0)

            qT_sb = sb.tile([128, 2, L], BF16, tag="qT")
            kT_sb = sb.tile([128, 2, L], BF16, tag="kT")
            v_sb = sb.tile([128, 16, H_LOC, D + 1], BF16, tag="v")
            oT_sb = sb.tile([128, 2, L], BF16, tag="oT")

            nc.vector.tensor_copy(
                v_sb[:, :, :, D:D + 1],
                ones_f.rearrange("p (a b c) -> p a b c", a=16, b=4),
            )

            # ---- projection / output emitters ------------------------------
            def emit_qk_chunk(w_sb, t_sb, m, n):
                # one [128 d-pair, 512 i] chunk of Q^T or K^T = W^T x^T
                p = pp.tile([128, 512], F32, tag="p", name="qk_ps")
                for k in range(KT):
                    nc.tensor.matmul(
                        p[:, :],
                        w_sb[:, k, m * 128:(m + 1) * 128],
                        xT_sb[:, k, n * 512:(n + 1) * 512],
                        start=(k == 0), stop=(k == KT - 1),
                    )
                nc.vector.tensor_copy(t_sb[:, m, n * 512:(n + 1) * 512], p[:, :])

            def emit_v_chunk(it):
                # V rows for j-tile `it`, all 4 heads at once (N=256)
                p = pp.tile([128, 512], F32, tag="p", name="v_ps")
                acc = p[:, 0:HD]
                for k in range(KT):
                    nc.tensor.matmul(
                        acc,
                        xT_sb[:, k, it * 128:(it + 1) * 128],
                        wv_sb[:, k, :],
                        start=(k == 0), stop=(k == KT - 1),
                    )
                nc.vector.tensor_copy(
                    v_sb[:, it, :, 0:D],
                    acc.rearrange("p (h d) -> p h d", h=H_LOC),
                )

            def emit_wo_chunk(ct, q):
                # [128 ct, 512 i] output chunk (contraction over HD=256)
                p = pp.tile([128, 512], F32, tag="p", name="wo_ps")
                for kk in range(2):
                    nc.tensor.matmul(
                        p[:, :],
                        wo_sb[:, kk, ct * 128:(ct + 1) * 128],
                        oT_sb[:, kk, q * 512:(q + 1) * 512],
                        start=(kk == 0), stop=(kk == 1),
                    )
                ost = ost_pool.tile([128, 512], F32, tag="ost", name="ost")
                nc.vector.tensor_copy(ost[:], p[:, :])
                nc.sync.dma_start(
                    outT[ct * 128:(ct + 1) * 128, q * 512:(q + 1) * 512], ost[:])

            # ---- deadline-driven work queue for PE slack -------------------
            # items: [deadline_step, cost_ns, emit_fn]; pumped each j-step
            work = []

            def pump(s, avail):
                while work and work[0][0] <= s:
                    _, c, f = work.pop(0)
                    f()
                    avail -= c
                while work and avail > 0:
                    idx = None
                    for i, (_, c, _f) in enumerate(work):
                        if c <= avail + 150.0:
                            idx = i
                            break
                    if idx is None:
                        break
                    _, c, f = work.pop(idx)
                    f()
                    avail -= c
                return avail

            QK_COST = 1750.0
            V_COST = 1000.0
            WO_COST = 500.0

            # Deadlines (in j-steps): every unit sweeps the FULL kT range
            # (j = 0..2048), so ALL kT chunks for pair m must be emitted well
            # before unit (m, 0) starts. qT chunk n is only read by unit
            # (m, q=n) -- give it ~8 steps of runway. V chunk `it` is read by
            # the AV emitted at step it+1.
            for it in range(8, 16):
                work.append([it - 1.0, V_COST,
                             lambda it=it: emit_v_chunk(it)])
            for n in range(2, 4):
                work.append([float(n) - 1.0, QK_COST,
                             lambda n=n: emit_qk_chunk(wk_sb, kT_sb, 0, n)])
            for n in range(1, 4):
                work.append([16.0 * n - 8.0, QK_COST,
                             lambda n=n: emit_qk_chunk(wq_sb, qT_sb, 0, n)])
            for i, n in enumerate(range(4)):
                work.append([40.0 + 4 * i, QK_COST,
                             lambda n=n: emit_qk_chunk(wk_sb, kT_sb, 1, n)])
            for n in range(4):
                work.append([56.0 + 16 * n, QK_COST,
                             lambda n=n: emit_qk_chunk(wq_sb, qT_sb, 1, n)])
            work.sort(key=lambda item: item[0])

            # ---- prologue: just q/k (m0, n0), chasing the n0 DMA stream ----
            emit_qk_chunk(wq_sb, qT_sb, 0, 0)
            emit_qk_chunk(wk_sb, kT_sb, 0, 0)
            emit_qk_chunk(wk_sb, kT_sb, 0, 1)
            for it in range(8):
                emit_v_chunk(it)

            # ---- attention units ------------------------------------------
            # finished units queue normalize closures; one runs per j-step.
            norm_steps = []

            def queue_normalize(m, q, o_cps):
                i0 = q * 512
                d0s = [None, None]

                def recip(hl):
                    # reshape the denominator row across all 128 lanes so the
                    # reciprocal runs at full DVE width, then reshape back
                    dsq = nrm_pool.tile([128, 4], F32, tag=f"dsq{hl}",
                                        name=f"dsq{hl}")
                    nc.gpsimd.dma_start(dsq[:], o_cps[hl][64:65, :])
                    nc.vector.reciprocal(dsq[:], dsq[:])
                    d0 = nrm_pool.tile([1, 512], F32, tag=f"d0_{hl}",
                                       name=f"d0_{hl}")
                    nc.gpsimd.dma_start(d0[:], dsq[:])
                    d0s[hl] = d0

                def scale(hl):
                    rep = nrm_pool.tile([64, 512], F32, tag=f"rep{hl}",
                                        name=f"rep{hl}")
                    nc.gpsimd.partition_broadcast(rep[:], d0s[hl][:])
                    with nc.allow_low_precision(reason="bf16 oT"):
                        if hl == 0:
                            nc.vector.tensor_mul(
                                oT_sb[0:64, m, i0:i0 + 512],
                                o_cps[hl][0:64, :], rep[:])
                        else:
                            stage = nrm_pool.tile([64, 512], BF16, tag="stage",
                                                  name="stage")
                            nc.vector.tensor_mul(
                                stage[:], o_cps[hl][0:64, :], rep[:])
                            nc.gpsimd.dma_start(
                                oT_sb[64:128, m, i0:i0 + 512], stage[:])

                norm_steps.append(lambda: recip(0))
                norm_steps.append(lambda: recip(1))
                norm_steps.append(lambda: scale(0))
                norm_steps.append(lambda: scale(1))
                if m == 1:
                    # oT for this i-block complete after both scales -> Wo
                    def queue_wo():
                        for ct in range(8):
                            work.append([10 ** 9, WO_COST,
                                         lambda ct=ct, q=q: emit_wo_chunk(ct, q)])
                    norm_steps.append(queue_wo)

            units = [(0, q) for q in range(4)] + [(1, q) for q in range(4)]
            for ui, (m, q) in enumerate(units):
                i0 = q * 512
                acc_h = [None, None]
                es_prev = None
                for j in range(16):
                    s = 16 * ui + j
                    # S^T pair -> supertile (adjacent pc: row groups 0 / 64)
                    spt = sp.tile([128, 1024], F32, tag="s", name="spt")
                    nc.tensor.matmul(
                        spt[:, 0:512],
                        kT_sb[0:64, m, j * 128:(j + 1) * 128],
                        qT_sb[0:64, m, i0:i0 + 512],
                        start=True, stop=True,
                    )
                    nc.tensor.matmul(
                        spt[:, 512:1024],
                        kT_sb[64:128, m, j * 128:(j + 1) * 128],
                        qT_sb[64:128, m, i0:i0 + 512],
                        start=True, stop=True,
                    )
                    es = es_pool.tile([128, 1024], BF16, tag="es", name="es")
                    nc.scalar.activation(es[:], spt[:], AF.Exp, scale=SCALE2)
                    # AV for step j-1 (one step behind)
                    if j > 0:
                        if j == 1:
                            acc_h[0] = op.tile([128, 512], F32, tag="o",
                                               name="acc0")
                            acc_h[1] = op.tile([128, 512], F32, tag="o",
                                               name="acc1")
                        for hl in range(2):
                            nc.tensor.matmul(
                                acc_h[hl][0:65, :],
                                v_sb[:, j - 1, 2 * m + hl, :],
                                es_prev[:, hl * 512:(hl + 1) * 512],
                                start=(j == 1), stop=False,
                            )
                    if norm_steps:
                        norm_steps.pop(0)()
                    pump(s, 750.0)
                    es_prev = es
                # epilogue AV for j=15
                for hl in range(2):
                    nc.tensor.matmul(
                        acc_h[hl][0:65, :],
                        v_sb[:, 15, 2 * m + hl, :],
                        es_prev[:, hl * 512:(hl + 1) * 512],
                        start=False, stop=True,
                    )
                # pull O' + denominator off PSUM so the banks free quickly
                o_cps = []
                for hl in range(2):
                    o_cp = ocp_pool.tile([65, 512], F32, tag=f"ocp{hl}",
                                         name=f"ocp{hl}")
                    nc.vector.tensor_copy(o_cp[:], acc_h[hl][0:65, :])
                    o_cps.append(o_cp)
                queue_normalize(m, q, o_cps)

            # ---- epilogue: last normalize + remaining Wo -------------------
            while norm_steps:
                norm_steps.pop(0)()
            while work:
                work.pop(0)[2]()

            ost_pool.release()
            nrm_pool.release()
            ocp_pool.release()
            es_pool.release()

    nc.compile()
    return nc


def kernel(x, Wq, Wk, Wv, Wo, bo):
    x = np.asarray(x, dtype=np.float32)
    Wq = np.asarray(Wq, dtype=np.float32)
    Wk = np.asarray(Wk, dtype=np.float32)
    Wv = np.asarray(Wv, dtype=np.float32)
    Wo = np.asarray(Wo, dtype=np.float32)
    bo = np.asarray(bo, dtype=np.float32)

    if "nc" not in _cache:
        _cache["nc"] = _build()
    nc = _cache["nc"]

    xTs = [np.ascontiguousarray(x[b].T) for b in range(B)]
    in_maps = []
    for core in range(8):
        b, hg = divmod(core, 4)
        sl = slice(hg * HD, (hg + 1) * HD)
        def pkc(w):
            # [(k p), c] -> [p, (k c)] so the on-chip DMA is contiguous
            kk, cc = w.shape[0] // 128, w.shape[1]
            return np.ascontiguousarray(
                w.reshape(kk, 128, cc).transpose(1, 0, 2).reshape(128, kk * cc)
            ).astype(ml_dtypes.bfloat16)

        in_maps.append({
            "xT": xTs[b].astype(ml_dtypes.bfloat16),
            "wq": pkc(Wq[:, sl]),
            "wk": pkc(Wk[:, sl]),
            "wv": pkc(Wv[:, sl]),
            "wo": pkc(Wo[sl, :]),
        })

    res = run_bass_kernel_spmd(nc, in_maps, core_ids=list(range(8)))
    out = np.empty((B, L, C), dtype=np.float32)
    for b in range(B):
        acc = res.results[4 * b]["outT"]
        for hg in range(1, 4):
            acc = acc + res.results[4 * b + hg]["outT"]
        out[b] = acc.T + bo
    return out
